# revision 2
# baseline (speedup 1.0000x reference)
"""GATv2 (3-layer, 4-head) message-passing kernel for Trainium2, 8-core SPMD.

Strategy (per sharding hint): nodes sharded contiguously across 8 cores;
edges partitioned by destination; per-layer AllGather of the source-side
transform xl = x @ Wl (bf16) so each core can gather arbitrary source rows;
segment softmax / scatter-add stay local per destination shard.

V4 (this file): all per-edge rows come from batched InstDMAGatherAnt row
gathers (<=1024 indices per call -- the single-packet 64-descriptor/engine
ring ceiling; single_packet=False is fatal on this runtime):
  - xl[src] rows from the AllGathered table, split lo (src < 32768) / hi
    (idx = src - (N - 32768)) to fit signed int16 indices,
  - xr[dst] rows from the local shard's xr table (dst_local < 32768).
With both per-edge operands gathered, m = xl[src] + xr[dst] is one bulk DVE
add; the attention weighted sum uses zee = exp(logit) * xl[src] directly, so
out = segsum(zee)/segsum(ee) with no xr correction term.

Segment reductions use a 0/1 selection matrix S[e, dst_local] built on-chip by
an is_equal compare against an iota row; one PE matmul per 128-edge k-tile
accumulates both the weighted feature sum and the softmax denominator
(rhs = [zee | ee], 132 columns) into PSUM.

ACT table discipline: every scalar-engine op (Copy, Exp, Ln, Prelu, Relu)
lives in the one "natural_log_exp_and_others" table set: leaky-relu runs as
Prelu, and LN's rsqrt(v) = exp(-0.5*ln(v)) -- no Sqrt, so no per-chunk
activation-table reloads.
"""

import sys

sys.path.insert(0, "/opt/trn_rl_repo")

import os

import ml_dtypes
import numpy as np

import concourse.bass as bass
import concourse.bacc as bacc
import concourse.tile as tile
from concourse import mybir

F32 = mybir.dt.float32
I32 = mybir.dt.int32
I16 = mybir.dt.int16
BF16 = mybir.dt.bfloat16
AF = mybir.ActivationFunctionType
ALU = mybir.AluOpType
AX = mybir.AxisListType

P = 128
NEG_SLOPE = 0.2
LN_EPS = 1e-5
DENOM_EPS = 1e-30
SPLIT = 32768  # int16 index ceiling for the lo gather table
GMAX = 8  # k-tiles per dma_gather call (1024 idx = 64-desc/engine packet cap)
ACT_PRELU = bool(int(os.environ.get("GAT_ACT_PRELU", "1")))
DBG_LAYERS = int(os.environ.get("GAT_LAYERS", "0"))  # 0 = all


class Cfg:
    def __init__(self, N=50000, D=128, H=4, L=3, n_cores=8):
        self.N, self.D, self.H, self.L, self.M = N, D, H, L, n_cores
        self.C = D // H
        assert N % n_cores == 0
        self.shard = N // n_cores
        self.chunks = (self.shard + P - 1) // P


# ----------------------------------------------------------------------------
# Host preprocessing: append self loops, sort by dst, pack per-core chunk
# gather-index / selection arrays.
# ----------------------------------------------------------------------------

def _wrap16(a):
    """Linear idx list [n] -> dma_gather layout [128, n/16] int16 (value for
    gathered row i sits at partition i%16, col i//16; replicated across the 8
    Q7 groups)."""
    n = len(a)
    assert n % 16 == 0
    a16 = a.reshape(-1, 16).T.astype(np.int16)  # [16, n/16]
    return np.ascontiguousarray(np.tile(a16, (8, 1)))


def preprocess(edge_index, cfg):
    N, M, shard, chunks = cfg.N, cfg.M, cfg.shard, cfg.chunks
    HIB = N - SPLIT  # hi-table base; hi idx = src - HIB in [N-2*SPLIT.., 32767]
    assert shard < SPLIT
    ei = np.asarray(edge_index)
    loops = np.arange(N, dtype=np.int64)
    src = np.concatenate([ei[0].astype(np.int64), loops])
    dst = np.concatenate([ei[1].astype(np.int64), loops])
    order = np.argsort(dst, kind="stable")
    src_s, dst_s = src[order], dst[order]

    per_core = []
    max_lo = max_hi = 1
    for c in range(M):
        lo, hi = np.searchsorted(dst_s, [c * shard, (c + 1) * shard])
        d_loc = dst_s[lo:hi] - c * shard
        s_loc = src_s[lo:hi]
        ch = d_loc // P
        chunk_edges = []
        for t in range(chunks):
            m = ch == t
            sl, dl = s_loc[m], d_loc[m]
            is_lo = sl < SPLIT
            chunk_edges.append((sl[is_lo], dl[is_lo], sl[~is_lo], dl[~is_lo]))
            max_lo = max(max_lo, -(-int(is_lo.sum()) // P))
            max_hi = max(max_hi, -(-int((~is_lo).sum()) // P))
        per_core.append(chunk_edges)

    KLO, KHI = max_lo, max_hi
    K = KLO + KHI
    meta = {"K": K, "KLO": KLO, "KHI": KHI, "HIB": HIB}

    pre = []
    for c in range(M):
        idx_lo = np.zeros((chunks, P, KLO * 8), dtype=np.int16)
        idx_hi = np.zeros((chunks, P, KHI * 8), dtype=np.int16)
        idx_xr = np.zeros((chunks, P, K * 8), dtype=np.int16)
        dstl = np.full((chunks, P, K), 300.0, dtype=np.float32)
        for t in range(chunks):
            sl_lo, dl_lo, sl_hi, dl_hi = per_core[c][t]
            lin_lo = np.zeros(KLO * P, dtype=np.int64)
            lin_lo[: len(sl_lo)] = sl_lo
            lin_hi = np.zeros(KHI * P, dtype=np.int64)
            lin_hi[: len(sl_hi)] = sl_hi - HIB
            lin_xr = np.zeros(K * P, dtype=np.int64)
            lin_xr[: len(dl_lo)] = dl_lo
            lin_xr[KLO * P : KLO * P + len(dl_hi)] = dl_hi
            idx_lo[t] = _wrap16(lin_lo)
            idx_hi[t] = _wrap16(lin_hi)
            idx_xr[t] = _wrap16(lin_xr)
            for (darr, k0) in ((dl_lo, 0), (dl_hi, KLO)):
                j = np.arange(len(darr))
                p, k = j % P, k0 + j // P
                dstl[t, p, k] = (darr - t * P).astype(np.float32)
        pre.append(
            {
                "idx_lo": idx_lo,
                "idx_hi": idx_hi,
                "idx_xr": idx_xr,
                "dstl16": dstl.astype(ml_dtypes.bfloat16),
            }
        )
    return pre, meta


def _groups(n):
    return [(a, min(a + GMAX, n)) for a in range(0, n, GMAX)]


# ----------------------------------------------------------------------------
# Kernel builder. io maps logical names to DRAM APs (ExternalInput/Output).
# ----------------------------------------------------------------------------

def build(tc, io, cfg, meta):
    from contextlib import ExitStack

    nc = tc.nc
    D, H, L, C = cfg.D, cfg.H, cfg.L, cfg.C
    K, KLO, KHI, HIB = meta["K"], meta["KLO"], meta["KHI"], meta["HIB"]
    shard, chunks = cfg.shard, cfg.chunks

    ctx = ExitStack()
    dram = ctx.enter_context(tc.tile_pool(name="drampool", bufs=1, space="DRAM"))
    consts = ctx.enter_context(tc.tile_pool(name="consts", bufs=1))
    lconsts = ctx.enter_context(tc.tile_pool(name="lconsts", bufs=2))
    nodep = ctx.enter_context(tc.tile_pool(name="nodep", bufs=3))
    idxp = ctx.enter_context(tc.tile_pool(name="idxp", bufs=3))
    edgep = ctx.enter_context(tc.tile_pool(name="edgep", bufs=3))
    smallp = ctx.enter_context(tc.tile_pool(name="smallp", bufs=3))
    ps_o = ctx.enter_context(tc.tile_pool(name="ps_o", bufs=2, space="PSUM"))
    ps_n = ctx.enter_context(tc.tile_pool(name="ps_n", bufs=3, space="PSUM"))
    ps_t = ctx.enter_context(tc.tile_pool(name="ps_t", bufs=2, space="PSUM"))

    # internal DRAM buffers (each tile its own tensor -> offset 0 for gathers)
    xl_sh = [dram.tile([shard, D], BF16, name=f"xl_sh{l}") for l in range(L)]
    xl_all = [
        dram.tile([cfg.N, D], BF16, name=f"xl_all{l}", addr_space="Shared")
        for l in range(L)
    ]
    xr_dram = [dram.tile([shard, D], BF16, name=f"xr_dram{l}") for l in range(L)]
    xst = [dram.tile([shard, D], F32, name=f"xst{l}") for l in range(L - 1)]
    xT = [dram.tile([P, chunks * P], F32, name=f"xT{l}") for l in range(L)]

    # constants resident in SBUF
    ident_sb = consts.tile([P, P], F32, name="ident_sb")
    nc.sync.dma_start(out=ident_sb[:], in_=io["ident"][:, :])
    iota16_sb = consts.tile([P, P], BF16, name="iota16_sb")
    nc.gpsimd.dma_start(out=iota16_sb[:], in_=_row_bcast(io["iota16"], 0, P, P))

    # ------------------------------------------------------------------
    # prologue: build xT[0] = transpose of x_shard
    # ------------------------------------------------------------------
    for t in range(chunks):
        nt = min(P, shard - t * P)
        xq0 = nodep.tile([P, D], F32, name="xq0")
        nc.sync.dma_start(out=xq0[:nt, :], in_=io["x_shard"][t * P : t * P + nt, :])
        psT = ps_t.tile([P, P], F32, name="psT", tag="psT")
        nc.tensor.transpose(
            out=psT[:, :nt], in_=xq0[:nt, :], identity=ident_sb[:nt, :nt]
        )
        sbT = nodep.tile([P, P], F32, name="sbT")
        nc.scalar.activation(out=sbT[:, :nt], in_=psT[:, :nt], func=AF.Copy)
        nc.sync.dma_start(out=xT[0][:, t * P : t * P + nt], in_=sbT[:, :nt])

    L_eff = DBG_LAYERS if DBG_LAYERS else L
    for l in range(L_eff):
        # per-layer constants (broadcast across partitions)
        wl_sb = lconsts.tile([P, D], F32, name="wl_sb")
        nc.sync.dma_start(out=wl_sb[:], in_=io["Wl"][l, :, :])
        wr_sb = lconsts.tile([P, D], F32, name="wr_sb")
        nc.sync.dma_start(out=wr_sb[:], in_=io["Wr"][l, :, :])
        attb_sb = lconsts.tile([P, D], BF16, name="attb_sb")
        nc.gpsimd.dma_start(out=attb_sb[:], in_=_row_bcast(io["attb16"], l, P, D))
        bc_sb = lconsts.tile([P, D], F32, name="bc_sb")
        nc.gpsimd.dma_start(out=bc_sb[:], in_=_row_bcast(io["bc"], l, P, D))
        cvec_sb = lconsts.tile([P, D], F32, name="cvec_sb")
        nc.gpsimd.dma_start(out=cvec_sb[:], in_=_row_bcast(io["cvec"], l, P, D))
        gamma_sb = lconsts.tile([P, D], F32, name="gamma_sb")
        nc.gpsimd.dma_start(out=gamma_sb[:], in_=_row_bcast(io["gamma"], l, P, D))
        beta_sb = lconsts.tile([P, D], F32, name="beta_sb")
        nc.gpsimd.dma_start(out=beta_sb[:], in_=_row_bcast(io["beta"], l, P, D))

        # --------------------------------------------------------------
        # node phase: xl = x@Wl (bf16), xr = x@Wr + (bl+br) (bf16)
        # --------------------------------------------------------------
        for t in range(chunks):
            nt = min(P, shard - t * P)
            lhsT = nodep.tile([P, P], F32, name="lhsT")
            nc.sync.dma_start(out=lhsT[:, :nt], in_=xT[l][:, t * P : t * P + nt])
            ps_xl = ps_n.tile([P, D], F32, name="ps_xl", tag="ps_n")
            nc.tensor.matmul(
                out=ps_xl[:nt, :], lhsT=lhsT[:, :nt], rhs=wl_sb[:], start=True, stop=True
            )
            xl_o = nodep.tile([P, D], BF16, name="xl_o")
            nc.scalar.activation(out=xl_o[:nt, :], in_=ps_xl[:nt, :], func=AF.Copy)
            nc.sync.dma_start(out=xl_sh[l][t * P : t * P + nt, :], in_=xl_o[:nt, :])

            ps_xr = ps_n.tile([P, D], F32, name="ps_xr", tag="ps_n")
            nc.tensor.matmul(
                out=ps_xr[:nt, :], lhsT=lhsT[:, :nt], rhs=wr_sb[:], start=True, stop=True
            )
            xr_o = nodep.tile([P, D], BF16, name="xr_o")
            nc.vector.tensor_tensor(
                out=xr_o[:nt, :], in0=ps_xr[:nt, :], in1=bc_sb[:nt, :], op=ALU.add
            )
            nc.sync.dma_start(out=xr_dram[l][t * P : t * P + nt, :], in_=xr_o[:nt, :])

        # --------------------------------------------------------------
        # AllGather xl across the 8 cores
        # --------------------------------------------------------------
        nc.gpsimd.collective_compute(
            "AllGather",
            ALU.bypass,
            replica_groups=[list(range(cfg.M))],
            ins=[xl_sh[l][:, :].opt()],
            outs=[xl_all[l][:, :].opt()],
        )

        # --------------------------------------------------------------
        # edge phase, one chunk of 128 destinations at a time
        # --------------------------------------------------------------
        for ch in range(chunks):
            nt = min(P, shard - ch * P)
            rows = slice(ch * P, ch * P + nt)

            dstl_sb = idxp.tile([P, K], BF16, name="dstl_sb")
            nc.sync.dma_start(out=dstl_sb[:], in_=io["dstl16"][ch, :, :])
            idxlo_sb = idxp.tile([P, KLO * 8], I16, name="idxlo_sb")
            nc.sync.dma_start(out=idxlo_sb[:], in_=io["idx_lo"][ch, :, :])
            idxhi_sb = idxp.tile([P, KHI * 8], I16, name="idxhi_sb")
            nc.sync.dma_start(out=idxhi_sb[:], in_=io["idx_hi"][ch, :, :])
            idxxr_sb = idxp.tile([P, K * 8], I16, name="idxxr_sb")
            nc.sync.dma_start(out=idxxr_sb[:], in_=io["idx_xr"][ch, :, :])

            # gathered per-edge rows: g2 = xl[src], xr_g = xr[dst]
            g2 = edgep.tile([P, K, D], BF16, name="g2")
            for a, b in _groups(KLO):
                nc.gpsimd.dma_gather(
                    out_ap=g2[:, a:b, :],
                    in_ap=xl_all[l][0:SPLIT, :],
                    idxs_ap=idxlo_sb[:, a * 8 : b * 8],
                    num_idxs=(b - a) * P,
                    num_idxs_reg=(b - a) * P,
                    elem_size=D,
                )
            for a, b in _groups(KHI):
                nc.gpsimd.dma_gather(
                    out_ap=g2[:, KLO + a : KLO + b, :],
                    in_ap=xl_all[l][HIB : cfg.N, :],
                    idxs_ap=idxhi_sb[:, a * 8 : b * 8],
                    num_idxs=(b - a) * P,
                    num_idxs_reg=(b - a) * P,
                    elem_size=D,
                )
            xr_g = edgep.tile([P, K, D], BF16, name="xr_g")
            for a, b in _groups(K):
                nc.gpsimd.dma_gather(
                    out_ap=xr_g[:, a:b, :],
                    in_ap=xr_dram[l][:, :],
                    idxs_ap=idxxr_sb[:, a * 8 : b * 8],
                    num_idxs=(b - a) * P,
                    num_idxs_reg=(b - a) * P,
                    elem_size=D,
                )

            # m = xl[src] + xr[dst]; leaky relu
            m_t = edgep.tile([P, K, D], BF16, name="m_t")
            nc.vector.tensor_tensor(
                out=m_t[:, :, :], in0=g2[:, :, :], in1=xr_g[:, :, :], op=ALU.add
            )
            lk = edgep.tile([P, K, D], BF16, name="lk")
            if ACT_PRELU:
                nc.scalar.activation(
                    out=lk[:, :, :], in_=m_t[:, :, :], func=AF.Prelu,
                    alpha=NEG_SLOPE,
                )
            else:
                nc.vector.tensor_scalar(
                    out=lk[:, :, :], in0=m_t[:, :, :], scalar1=NEG_SLOPE,
                    scalar2=None, op0=ALU.mult,
                )
                nc.vector.tensor_tensor(
                    out=lk[:, :, :], in0=lk[:, :, :], in1=m_t[:, :, :], op=ALU.max
                )

            # attention logits, exp; zee = [ee * xl[src] | ee]
            tt = edgep.tile([P, K, D], BF16, name="tt")
            nc.vector.tensor_tensor(
                out=tt[:, :, :],
                in0=lk[:, :, :],
                in1=attb_sb[:, :].unsqueeze(1).to_broadcast([P, K, D]),
                op=ALU.mult,
            )
            lg = smallp.tile([P, K, H], F32, name="lg")
            nc.vector.reduce_sum(
                out=lg[:, :, :],
                in_=tt[:, :, :].rearrange("p k (h c) -> p k h c", h=H),
                axis=AX.X,
            )
            zee = edgep.tile([P, K, D + H], BF16, name="zee")
            nc.scalar.activation(out=zee[:, :, D : D + H], in_=lg[:, :, :], func=AF.Exp)
            nc.vector.tensor_tensor(
                out=zee[:, :, 0:D].rearrange("p k (h c) -> p k h c", h=H),
                in0=g2[:, :, :].rearrange("p k (h c) -> p k h c", h=H),
                in1=zee[:, :, D : D + H].unsqueeze(3).to_broadcast([P, K, H, C]),
                op=ALU.mult,
            )

            # selection matrix S[e, dst_local]
            S = edgep.tile([P, K, P], BF16, name="S")
            nc.vector.tensor_tensor(
                out=S[:, :, :],
                in0=dstl_sb[:, :].unsqueeze(2).to_broadcast([P, K, P]),
                in1=iota16_sb[:, :].unsqueeze(1).to_broadcast([P, K, P]),
                op=ALU.is_equal,
            )

            # segment sums on PE: psum[dst, 0:D] = sum ee*xl ; psum[dst, D:] = denom
            po = ps_o.tile([P, D + H], F32, name="po")
            for k in range(K):
                nc.tensor.matmul(
                    out=po[:, :],
                    lhsT=S[:, k, :],
                    rhs=zee[:, k, :],
                    start=(k == 0),
                    stop=(k == K - 1),
                )

            dn = smallp.tile([P, H], F32, name="dn")
            nc.vector.tensor_scalar(
                out=dn[:, :], in0=po[:, D : D + H], scalar1=DENOM_EPS, scalar2=None,
                op0=ALU.add,
            )
            rd = smallp.tile([P, H], F32, name="rd")
            nc.vector.reciprocal(out=rd[:, :], in_=dn[:, :])

            onrm = smallp.tile([P, D], F32, name="onrm")
            nc.vector.tensor_tensor(
                out=onrm[:, :].rearrange("p (h c) -> p h c", h=H),
                in0=po[:, 0:D].rearrange("p (h c) -> p h c", h=H),
                in1=rd[:, :].unsqueeze(2).to_broadcast([P, H, C]),
                op=ALU.mult,
            )

            # h = onrm + (bl + gat_bias); then residual + LN
            xq = smallp.tile([P, D], F32, name="xq")
            if l == 0:
                nc.sync.dma_start(out=xq[:nt, :], in_=io["x_shard"][rows, :])
            else:
                nc.sync.dma_start(out=xq[:nt, :], in_=xst[l - 1][rows, :])

            t2 = smallp.tile([P, D], F32, name="t2")
            nc.vector.tensor_tensor(
                out=t2[:nt, :], in0=onrm[:nt, :], in1=cvec_sb[:nt, :], op=ALU.add
            )
            t3 = smallp.tile([P, D], F32, name="t3")
            nc.vector.tensor_tensor(
                out=t3[:nt, :], in0=t2[:nt, :], in1=xq[:nt, :], op=ALU.add
            )

            st6 = smallp.tile([P, 6], F32, name="st6")
            nc.vector.bn_stats(out=st6[:nt, :], in_=t3[:nt, :])
            mv = smallp.tile([P, 2], F32, name="mv")
            nc.vector.bn_aggr(out=mv[:nt, :], in_=st6[:nt, :])
            veps = smallp.tile([P, 1], F32, name="veps")
            nc.vector.tensor_scalar(
                out=veps[:nt, :], in0=mv[:nt, 1:2], scalar1=LN_EPS, scalar2=None,
                op0=ALU.add,
            )
            # rstd = 1/sqrt(veps) = exp(-0.5*ln(veps)) -- stays in the exp/ln
            # activation-table set (no Sqrt table reload per chunk)
            lnv = smallp.tile([P, 1], F32, name="lnv")
            nc.scalar.activation(out=lnv[:nt, :], in_=veps[:nt, :], func=AF.Ln)
            rstd = smallp.tile([P, 1], F32, name="rstd")
            nc.scalar.activation(
                out=rstd[:nt, :], in_=lnv[:nt, :], func=AF.Exp, scale=-0.5
            )

            y1 = smallp.tile([P, D], F32, name="y1")
            nc.vector.tensor_scalar(
                out=y1[:nt, :], in0=t3[:nt, :], scalar1=mv[:nt, 0:1],
                scalar2=rstd[:nt, :], op0=ALU.subtract, op1=ALU.mult,
            )
            y2 = smallp.tile([P, D], F32, name="y2")
            nc.vector.tensor_tensor(
                out=y2[:nt, :], in0=y1[:nt, :], in1=gamma_sb[:nt, :], op=ALU.mult
            )
            y3 = smallp.tile([P, D], F32, name="y3")
            nc.vector.tensor_tensor(
                out=y3[:nt, :], in0=y2[:nt, :], in1=beta_sb[:nt, :], op=ALU.add
            )

            if l < L_eff - 1:
                xo = smallp.tile([P, D], F32, name="xo")
                nc.scalar.activation(out=xo[:nt, :], in_=y3[:nt, :], func=AF.Relu)
                nc.sync.dma_start(out=xst[l][rows, :], in_=xo[:nt, :])
                psT2 = ps_t.tile([P, P], F32, name="psT2", tag="psT")
                nc.tensor.transpose(
                    out=psT2[:, :nt], in_=xo[:nt, :], identity=ident_sb[:nt, :nt]
                )
                sbT2 = smallp.tile([P, P], F32, name="sbT2")
                nc.scalar.activation(out=sbT2[:, :nt], in_=psT2[:, :nt], func=AF.Copy)
                nc.sync.dma_start(
                    out=xT[l + 1][:, ch * P : ch * P + nt], in_=sbT2[:, :nt]
                )
            else:
                nc.sync.dma_start(out=io["y"][rows, :], in_=y3[:nt, :])

    ctx.close()


def _row_bcast(ap, row, parts, d):
    """AP reading row `row` of a [R, 1, D] or [R, D] DRAM tensor, replicated
    across `parts` partitions (partition step 0)."""
    flat = ap[row] if ap.ndim == 3 else ap[row : row + 1]
    base = flat.opt()
    return bass.AP(tensor=base.tensor, offset=row * d, ap=[[0, parts], [1, d]])


# ----------------------------------------------------------------------------
# host-side inputs
# ----------------------------------------------------------------------------

def make_host_inputs(inputs, cfg):
    L, D, H, C = cfg.L, cfg.D, cfg.H, cfg.C
    bl = np.asarray(inputs["bl"], np.float32)
    br = np.asarray(inputs["br"], np.float32)
    att = np.asarray(inputs["att"], np.float32)
    gat_bias = np.asarray(inputs["bias"], np.float32)
    return {
        "Wl": np.asarray(inputs["Wl"], np.float32),
        "Wr": np.asarray(inputs["Wr"], np.float32),
        "attb16": att.reshape(L, 1, H * C).astype(ml_dtypes.bfloat16),
        "bc": (bl + br).reshape(L, 1, D),
        "cvec": (bl + gat_bias).reshape(L, 1, D),
        "gamma": np.asarray(inputs["gamma"], np.float32).reshape(L, 1, D),
        "beta": np.asarray(inputs["beta"], np.float32).reshape(L, 1, D),
        "ident": np.eye(P, dtype=np.float32),
        "iota16": np.arange(P, dtype=np.float32)
        .reshape(1, P)
        .astype(ml_dtypes.bfloat16),
    }


def make_in_maps(inputs, pre, cfg):
    x = np.asarray(inputs["fine_poi_x"], np.float32)
    shared = make_host_inputs(inputs, cfg)
    in_maps = []
    for c in range(cfg.M):
        m = dict(shared)
        m["x_shard"] = np.ascontiguousarray(
            x[c * cfg.shard : (c + 1) * cfg.shard]
        )
        for k in ("idx_lo", "idx_hi", "idx_xr", "dstl16"):
            m[k] = pre[c][k]
        in_maps.append(m)
    return in_maps


# ----------------------------------------------------------------------------
# program assembly + execution
# ----------------------------------------------------------------------------

_CACHE = {}


def _build_program(cfg, meta):
    K, KLO, KHI = meta["K"], meta["KLO"], meta["KHI"]
    key = (cfg.N, cfg.D, cfg.H, cfg.L, cfg.M, K, KLO)
    if key in _CACHE:
        return _CACHE[key]
    nc = bacc.Bacc(
        "TRN2", target_bir_lowering=False, debug=False, num_devices=cfg.M
    )
    io = {}
    io["x_shard"] = nc.dram_tensor(
        "x_shard", [cfg.shard, cfg.D], F32, kind="ExternalInput"
    ).ap()
    io["idx_lo"] = nc.dram_tensor(
        "idx_lo", [cfg.chunks, P, KLO * 8], I16, kind="ExternalInput"
    ).ap()
    io["idx_hi"] = nc.dram_tensor(
        "idx_hi", [cfg.chunks, P, KHI * 8], I16, kind="ExternalInput"
    ).ap()
    io["idx_xr"] = nc.dram_tensor(
        "idx_xr", [cfg.chunks, P, K * 8], I16, kind="ExternalInput"
    ).ap()
    io["dstl16"] = nc.dram_tensor(
        "dstl16", [cfg.chunks, P, K], BF16, kind="ExternalInput"
    ).ap()
    io["attb16"] = nc.dram_tensor(
        "attb16", [cfg.L, 1, cfg.D], BF16, kind="ExternalInput"
    ).ap()
    io["iota16"] = nc.dram_tensor("iota16", [1, P], BF16, kind="ExternalInput").ap()
    io["Wl"] = nc.dram_tensor(
        "Wl", [cfg.L, cfg.D, cfg.D], F32, kind="ExternalInput"
    ).ap()
    io["Wr"] = nc.dram_tensor(
        "Wr", [cfg.L, cfg.D, cfg.D], F32, kind="ExternalInput"
    ).ap()
    for nm in ["bc", "cvec", "gamma", "beta"]:
        io[nm] = nc.dram_tensor(
            nm, [cfg.L, 1, cfg.D], F32, kind="ExternalInput"
        ).ap()
    io["ident"] = nc.dram_tensor("ident", [P, P], F32, kind="ExternalInput").ap()
    io["y"] = nc.dram_tensor(
        "y", [cfg.shard, cfg.D], F32, kind="ExternalOutput"
    ).ap()

    with tile.TileContext(nc) as tc:
        build(tc, io, cfg, meta)
    nc.compile()
    _CACHE[key] = nc
    return nc


def kernel(**inputs):
    from concourse import bass_utils

    cfg = Cfg()
    pre, meta = preprocess(inputs["edge_index"], cfg)
    nc = _build_program(cfg, meta)
    in_maps = make_in_maps(inputs, pre, cfg)
    res = bass_utils.run_bass_kernel_spmd(
        nc, in_maps, core_ids=list(range(cfg.M))
    )
    out = np.concatenate([res.results[c]["y"] for c in range(cfg.M)], axis=0)
    return out.astype(np.float32)
